# revision 48
# baseline (speedup 1.0000x reference)
"""Trainium2 Bass kernel for GQA attention with RoPE (dense transformer block).

Strategy (8-way tensor parallel over heads, per the sharding hint):
  - Each of the 8 NeuronCores gets 4 Q heads + 1 KV head (KV head c, Q heads
    4c..4c+3); host sums the 8 partial bf16 outputs (the "all-reduce after wo").
  - All matmul operands are bf16 (fp32 PSUM accumulation) - same PE rate as
    fp32r in practice, half the DMA/SBUF footprint. Host pre-packs weights so
    every DMA is wide and contiguous.
  - Weights, Q^T, K^T, V token-major and the current output chunk are fully
    SBUF-resident; no DRAM roundtrips for activations.
  - Causal attention computes the exact 128x512 block triangle. Softmax is
    unnormalized exp (no max subtraction); the denominators come from two
    alternating elementwise accumulators (DVE) + one GPSIMD partition_all_reduce
    per (batch, head, 512-query chunk) - no PE row-sum matmuls.
  - Causal masking multiplies by a single SBUF-resident 512x512 diagonal
    block (the relative pattern is identical for every diagonal chunk).
  - The wo projection of each 512-token chunk is emitted with a one-group
    delay so its matmuls fill the PE gaps left by the scores->exp->PV
    dependency chain; outputs stream straight from PSUM through ACT/DVE
    copies into bf16 staging and out via two wide DMAs per token tile.
  - B-phase SBUF pools are opened before the phase-A scratch pools so the
    first attention tiles never alias scratch whose release depends on the
    last RoPE chain.
"""
import math

import numpy as np
import ml_dtypes

import concourse.bass as bass
import concourse.tile as tile
from concourse import bacc, bass_isa, mybir
from concourse.bass_utils import run_bass_kernel_spmd
from concourse.masks import make_identity

B, S, DIM = 2, 2048, 4096
NH, NKV, HD = 32, 8, 128
BS = B * S
NCORES = 8
QH = NH // NCORES          # 4 Q heads per core
DQ = QH * HD               # 512
TCH = 512                  # token chunk
NCH = BS // TCH            # 8 chunks
NKT = DIM // 128           # 32 contraction tiles
P = 128

F32 = mybir.dt.float32
F32R = mybir.dt.float32r
BF16 = mybir.dt.bfloat16
AF = mybir.ActivationFunctionType
NPBF = ml_dtypes.bfloat16

_prog_cache = {}
LAST_RESULTS = None


def _build(variant):
    """variant: 'causal' | 'none' | 'general'"""
    nc = bacc.Bacc(None, target_bir_lowering=False)
    xT = nc.dram_tensor("xT", [DIM, BS], BF16, kind="ExternalInput")
    wq = nc.dram_tensor("wq", [P, QH * NKT * HD], BF16, kind="ExternalInput")
    wk = nc.dram_tensor("wk", [P, NKT * HD], BF16, kind="ExternalInput")
    wv = nc.dram_tensor("wv", [P, NKT * HD], BF16, kind="ExternalInput")
    wo = nc.dram_tensor("wo", [DQ, DIM], BF16, kind="ExternalInput")
    cosT = nc.dram_tensor("cosT", [64, S], BF16, kind="ExternalInput")
    sinT = nc.dram_tensor("sinT", [64, S], BF16, kind="ExternalInput")
    mblk = None
    emask = None
    if variant == "causal":
        mblk = nc.dram_tensor("mblk", [4 * P, TCH], BF16, kind="ExternalInput")
    elif variant == "general":
        emask = nc.dram_tensor("emaskT", [S, S], BF16, kind="ExternalInput")
    part = nc.dram_tensor("part", [BS, DIM], BF16, kind="ExternalOutput")

    with tile.TileContext(nc) as tc:
        with (
            tc.tile_pool(name="const", bufs=1) as constp,
            tc.tile_pool(name="big", bufs=1) as bigp,
        ):
            ident = constp.tile([P, P], BF16)
            make_identity(nc, ident)
            cos_sb = constp.tile([64, S], BF16)
            sin_sb = constp.tile([64, S], BF16)
            msk = []
            if variant == "causal":
                for j in range(4):
                    mj = constp.tile([P, TCH], BF16, name=f"msk{j}")
                    msk.append(mj)

            # persistent per-batch activations (bf16)
            KT_sb = [bigp.tile([P, S], BF16, name=f"KT{b}") for b in range(B)]
            Vtok = [bigp.tile([P, S], BF16, name=f"Vtok{b}") for b in range(B)]
            qt_sb = [[bigp.tile([P, S], BF16, name=f"qt{h}_{b}")
                      for b in range(B)] for h in range(QH)]
            w_sb = [bigp.tile([P, NKT * HD], BF16, name=f"w{m}")
                    for m in range(6)]
            wo_sb = [bigp.tile([P, DIM], BF16, name=f"wo{kk}")
                     for kk in range(4)]

            wsrc = [wq[:, m * NKT * HD:(m + 1) * NKT * HD] for m in range(QH)]
            wsrc += [wk[:, :], wv[:, :]]
            C1 = 4 * HD

            # B-phase SBUF pools open FIRST so their tiles never alias the
            # phase-A scratch (whose release depends on the tcn7 rope tail)
            ebp_cm = tc.tile_pool(name="ebp", bufs=1)
            ebp = ebp_cm.__enter__()
            mkp_cm = tc.tile_pool(name="mkp", bufs=1)
            mkp = mkp_cm.__enter__()
            obp_cm = tc.tile_pool(name="obp", bufs=1)
            obp = obp_cm.__enter__()

            # ---------------- Phase A: QKV projection + RoPE ----------------
            with (
                tc.tile_pool(name="xtp", bufs=1) as xtp,
                tc.tile_pool(name="rp", bufs=1) as rp,
                tc.tile_pool(name="psA", bufs=1, space="PSUM") as psA,
            ):
                for tcn in range(NCH):
                    b, cb = divmod(tcn, NCH // B)
                    acc = [psA.tile([P, TCH], F32, tag="acc", bufs=7,
                                    name=f"acc{m}_{tcn}") for m in range(6)]
                    for k in range(NKT):
                        xt = xtp.tile([P, TCH], BF16, tag="xt", bufs=8,
                                      name=f"xt_{tcn}_{k}")
                        nc.sync.dma_start(
                            xt[:], xT[k * P:(k + 1) * P,
                                      tcn * TCH:(tcn + 1) * TCH])
                        if tcn == 0 and k == 0:
                            for m in range(6):
                                eng = nc.sync if m % 2 == 0 else nc.gpsimd
                                eng.dma_start(w_sb[m][:, 0:C1],
                                              wsrc[m][:, 0:C1])
                        if tcn == 0 and k == 22:
                            nc.gpsimd.dma_start(cos_sb[:], cosT[:, :])
                            nc.gpsimd.dma_start(sin_sb[:], sinT[:, :])
                            if variant == "causal":
                                for j in range(4):
                                    nc.gpsimd.dma_start(
                                        msk[j][:], mblk[j * P:(j + 1) * P, :])
                        if tcn == 0 and k in (0, 3, 6, 12):
                            lo = (4 + min(k // 3, 3) * 7) * HD
                            hi = min((4 + (min(k // 3, 3) + 1) * 7) * HD, NKT * HD)
                            for m in range(6):
                                nc.gpsimd.dma_start(w_sb[m][:, lo:hi],
                                                    wsrc[m][:, lo:hi])

                        for m in range(6):
                            nc.tensor.matmul(
                                acc[m][:], w_sb[m][:, k * HD:(k + 1) * HD],
                                xt[:], start=(k == 0), stop=(k == NKT - 1))

                    cs = cos_sb[:, cb * TCH:(cb + 1) * TCH]
                    sn = sin_sb[:, cb * TCH:(cb + 1) * TCH]
                    vch = rp.tile([P, TCH], BF16, tag="vch", bufs=2,
                                  name=f"vch_{tcn}")
                    nc.scalar.copy(vch[:], acc[5][:])
                    slos, shis = [], []
                    for m in range(5):
                        slo = rp.tile([64, TCH], BF16, tag="slo", bufs=3,
                                      name=f"slo{m}_{tcn}")
                        shi = rp.tile([64, TCH], BF16, tag="shi", bufs=3,
                                      name=f"shi{m}_{tcn}")
                        nc.scalar.copy(slo[:], acc[m][0:64, :])
                        nc.vector.tensor_copy(shi[:], acc[m][64:P, :])
                        slos.append(slo)
                        shis.append(shi)
                    tp4 = psA.tile([P, 4 * P], BF16, tag="tp", bufs=1,
                                   name=f"tp_{tcn}")
                    for j in range(TCH // P):
                        nc.tensor.transpose(
                            tp4[:, j * P:(j + 1) * P],
                            vch[:, j * P:(j + 1) * P], ident[:])
                    nc.vector.tensor_copy(
                        Vtok[b][:, cb * TCH:(cb + 1) * TCH], tp4[:])
                    for m in range(5):
                        slo, shi = slos[m], shis[m]
                        dst = qt_sb[m][b] if m < QH else KT_sb[b]
                        o_lo = dst[0:64, cb * TCH:(cb + 1) * TCH]
                        o_hi = dst[64:P, cb * TCH:(cb + 1) * TCH]
                        tA = rp.tile([64, TCH], BF16, tag="tA", bufs=2,
                                     name=f"tA{m}_{tcn}")
                        tB = rp.tile([64, TCH], BF16, tag="tB", bufs=2,
                                     name=f"tB{m}_{tcn}")
                        nc.vector.tensor_mul(tA[:], slo[:], cs)
                        nc.vector.tensor_mul(tB[:], shi[:], sn)
                        nc.vector.tensor_sub(o_lo, tA[:], tB[:])
                        tC = rp.tile([64, TCH], BF16, tag="tC", bufs=2,
                                     name=f"tC{m}_{tcn}")
                        tD = rp.tile([64, TCH], BF16, tag="tD", bufs=2,
                                     name=f"tD{m}_{tcn}")
                        nc.vector.tensor_mul(tC[:], slo[:], sn)
                        nc.vector.tensor_mul(tD[:], shi[:], cs)
                        nc.vector.tensor_add(o_hi, tC[:], tD[:])


            # ------------- Phase B+C merged per (batch, chunk) -------------
            with (
                tc.tile_pool(name="psB", bufs=1, space="PSUM") as psB,
            ):
                for kk in range(4):
                    nc.sync.dma_start(wo_sb[kk][:],
                                      wo[kk * P:(kk + 1) * P, :])
                pending_wo = []
                for b in range(B):
                    for sc in (0, 1, 2, 3):
                        ntt = 4 * sc + 4 if variant == "causal" else 16
                        o_g = [obp.tile([P, TCH], BF16, tag=f"og{h}",
                                        bufs=3, name=f"og_{b}_{sc}_{h}")
                               for h in range(QH)]
                        while len(pending_wo) > 1:
                            pending_wo.pop(0)()
                        for h in range(QH):
                            o_ps = psB.tile([P, TCH], F32, tag="o", bufs=2,
                                            name=f"o_{b}_{sc}_{h}")
                            E_ab = [ebp.tile([P, TCH], BF16, tag=f"ea{par}",
                                             bufs=2,
                                             name=f"ea{par}_{b}_{sc}_{h}")
                                    for par in range(2)]
                            for tt in range(ntt):
                                sc_ps = psB.tile([P, TCH], F32, tag="sc",
                                                 bufs=3,
                                                 name=f"s_{b}_{sc}_{h}_{tt}")
                                nc.tensor.matmul(
                                    sc_ps[:],
                                    KT_sb[b][:, tt * P:(tt + 1) * P],
                                    qt_sb[h][b][:, sc * TCH:(sc + 1) * TCH],
                                    start=True, stop=True)
                                masked = (variant == "general") or (
                                    variant == "causal" and tt >= 4 * sc)
                                et = ebp.tile([P, TCH], BF16, tag="et",
                                              bufs=6,
                                              name=f"et_{b}_{sc}_{h}_{tt}")
                                if masked:
                                    etm = ebp.tile(
                                        [P, TCH], BF16, tag="etm", bufs=2,
                                        name=f"em_{b}_{sc}_{h}_{tt}")
                                    nc.scalar.activation(etm[:], sc_ps[:],
                                                         AF.Exp)
                                    if variant == "causal":
                                        mt = msk[tt - 4 * sc][:]
                                    else:
                                        mg = mkp.tile(
                                            [P, TCH], BF16, tag="mg", bufs=3,
                                            name=f"mg_{b}_{sc}_{h}_{tt}")
                                        nc.sync.dma_start(
                                            mg[:],
                                            emask[tt * P:(tt + 1) * P,
                                                  sc * TCH:(sc + 1) * TCH])
                                        mt = mg[:]
                                    nc.vector.tensor_mul(et[:], etm[:], mt)
                                else:
                                    nc.scalar.activation(et[:], sc_ps[:],
                                                         AF.Exp)
                                ea = E_ab[tt % 2]
                                if tt < 2:
                                    nc.vector.tensor_copy(ea[:], et[:])
                                else:
                                    nc.vector.tensor_add(ea[:], ea[:], et[:])
                                nc.tensor.matmul(
                                    o_ps[:], Vtok[b][:, tt * P:(tt + 1) * P],
                                    et[:], start=(tt == 0),
                                    stop=(tt == ntt - 1))
                            e_sum = ebp.tile([P, TCH], BF16, tag="es",
                                             bufs=1, name=f"es_{b}_{sc}_{h}")
                            nc.vector.tensor_add(e_sum[:], E_ab[0][:],
                                                 E_ab[1][:])
                            srec = obp.tile([P, TCH], F32, tag="sr", bufs=1,
                                            name=f"sr_{b}_{sc}_{h}")
                            nc.gpsimd.partition_all_reduce(
                                srec[:], e_sum[:], P, bass_isa.ReduceOp.add)
                            rec = obp.tile([P, TCH], F32, tag="rec", bufs=1,
                                           name=f"rec_{b}_{sc}_{h}")
                            nc.vector.reciprocal(rec[:], srec[:])
                            nc.vector.tensor_mul(o_g[h][:], o_ps[:], rec[:])

                        # wo projection for this chunk, deferred one group
                        def make_wo(b=b, sc=sc, o_g=o_g):
                            def emit():
                                for tj in range(4):
                                    tt = 4 * sc + tj
                                    for half in range(2):
                                        ob = obp.tile(
                                            [P, DIM // 2], BF16, tag="ob",
                                            bufs=3,
                                            name=f"ob_{b}_{tt}_{half}")
                                        for nj in range(4):
                                            nn = half * 4 + nj
                                            pp = psB.tile(
                                                [P, TCH], F32, tag="pp",
                                                bufs=3,
                                                name=f"pp_{b}_{tt}_{nn}")
                                            for kk in range(4):
                                                nc.tensor.matmul(
                                                    pp[:],
                                                    o_g[kk][:,
                                                            tj * P:
                                                            (tj + 1) * P],
                                                    wo_sb[kk][:,
                                                              nn * TCH:
                                                              (nn + 1) * TCH],
                                                    start=(kk == 0),
                                                    stop=(kk == 3))
                                            dst = ob[:, nj * TCH:
                                                     (nj + 1) * TCH]
                                            if nn % 2 == 0:
                                                nc.scalar.copy(dst, pp[:])
                                            else:
                                                nc.vector.tensor_copy(dst,
                                                                      pp[:])
                                        nc.sync.dma_start(
                                            part[b * S + tt * P:
                                                 b * S + (tt + 1) * P,
                                                 half * (DIM // 2):
                                                 (half + 1) * (DIM // 2)],
                                            ob[:])
                            return emit
                        pending_wo.append(make_wo())
                for fn_ in pending_wo:
                    fn_()
                pending_wo = []

                obp_cm.__exit__(None, None, None)
                mkp_cm.__exit__(None, None, None)
                ebp_cm.__exit__(None, None, None)

    nc.compile()
    return nc


def _get_prog(variant):
    if variant not in _prog_cache:
        _prog_cache[variant] = _build(variant)
    return _prog_cache[variant]


def prepare(inputs):
    """Host-side sharding prep: returns (variant, program, per-core input maps)."""
    x = np.asarray(inputs["x"], dtype=np.float32)
    wq = np.asarray(inputs["wq"], dtype=np.float32)
    wk = np.asarray(inputs["wk"], dtype=np.float32)
    wv = np.asarray(inputs["wv"], dtype=np.float32)
    wo = np.asarray(inputs["wo"], dtype=np.float32)
    fc = np.asarray(inputs["freqs_cos"], dtype=np.float32)
    fs = np.asarray(inputs["freqs_sin"], dtype=np.float32)
    mask = np.asarray(inputs["mask"], dtype=np.float32)

    xT = np.ascontiguousarray(x.reshape(BS, DIM).T).astype(NPBF)
    perm = np.concatenate([np.arange(0, HD, 2), np.arange(1, HD, 2)])
    wq_p = (wq.reshape(DIM, NH, HD)[:, :, perm] / math.sqrt(HD))
    wk_p = wk.reshape(DIM, NKV, HD)[:, :, perm]
    cosT = np.ascontiguousarray(fc.T).astype(NPBF)
    sinT = np.ascontiguousarray(fs.T).astype(NPBF)

    if not mask.any():
        variant = "none"
    else:
        il, jl = np.tril_indices(S)
        iu, ju = np.triu_indices(S, 1)
        if np.all(mask[il, jl] == 0.0) and np.all(mask[iu, ju] <= -1e8):
            variant = "causal"
        else:
            variant = "general"

    mblk = None
    emaskT = None
    if variant == "causal":
        t = np.arange(4 * P)[:, None]
        q = np.arange(TCH)[None, :]
        mblk = (q >= t).astype(NPBF)
    elif variant == "general":
        with np.errstate(under="ignore", over="ignore"):
            emaskT = np.ascontiguousarray(np.exp(mask).T).astype(NPBF)

    nc = _get_prog(variant)

    in_maps = []
    for c in range(NCORES):
        wqc = wq_p[:, c * QH:(c + 1) * QH, :]                    # [DIM,QH,HD]
        wqc = np.ascontiguousarray(
            wqc.reshape(NKT, P, QH, HD).transpose(1, 2, 0, 3)
            .reshape(P, QH * NKT * HD)).astype(NPBF)
        wkc = np.ascontiguousarray(
            wk_p[:, c, :].reshape(NKT, P, HD).transpose(1, 0, 2)
            .reshape(P, NKT * HD)).astype(NPBF)
        wvc = np.ascontiguousarray(
            wv[:, c * HD:(c + 1) * HD].reshape(NKT, P, HD).transpose(1, 0, 2)
            .reshape(P, NKT * HD)).astype(NPBF)
        m = {
            "xT": xT,
            "wq": wqc,
            "wk": wkc,
            "wv": wvc,
            "wo": np.ascontiguousarray(
                wo[c * DQ:(c + 1) * DQ, :]).astype(NPBF),
            "cosT": cosT,
            "sinT": sinT,
        }
        if variant == "causal":
            m["mblk"] = mblk
        elif variant == "general":
            m["emaskT"] = emaskT
        in_maps.append(m)
    return variant, nc, in_maps


def kernel(**inputs):
    global LAST_RESULTS
    variant, nc, in_maps = prepare(inputs)
    out = None
    for attempt in range(3):
        res = run_bass_kernel_spmd(nc, in_maps, core_ids=list(range(NCORES)))
        LAST_RESULTS = res
        out = np.zeros((BS, DIM), dtype=np.float64)
        ok = True
        for c in range(NCORES):
            p = np.asarray(res.results[c]["part"], dtype=np.float64)
            # flaky-execution guard: a healthy partial is finite, nonzero,
            # and O(1)-scale; garbage shows up as huge values or all zeros
            if not np.isfinite(p).all() or np.abs(p).max() > 1e3 \
                    or np.abs(p).max() == 0.0:
                ok = False
            out += p
        if ok:
            break
    return out.reshape(B, S, DIM).astype(np.float32)


# revision 64
# speedup vs baseline: 1.0131x; 1.0131x over previous
"""Trainium2 Bass kernel for GQA attention with RoPE (dense transformer block).

Strategy (8-way tensor parallel over heads, per the sharding hint):
  - Each of the 8 NeuronCores gets 4 Q heads + 1 KV head (KV head c, Q heads
    4c..4c+3); host sums the 8 partial bf16 outputs (the "all-reduce after wo").
  - All matmul operands are bf16 (fp32 PSUM accumulation) - same PE rate as
    fp32r in practice, half the DMA/SBUF footprint. Host pre-packs weights so
    every DMA is wide and contiguous.
  - Weights, Q^T, K^T, V token-major and the current output chunk are fully
    SBUF-resident; no DRAM roundtrips for activations.
  - Causal attention computes the exact 128x512 block triangle. Softmax is
    unnormalized exp (no max subtraction); the denominators come from two
    alternating elementwise accumulators (DVE) + one GPSIMD partition_all_reduce
    per (batch, head, 512-query chunk) - no PE row-sum matmuls.
  - Causal masking multiplies by a single SBUF-resident 512x512 diagonal
    block (the relative pattern is identical for every diagonal chunk).
  - The wo projection is split into per-token-tile closures emitted with a
    ~one-group delay, one at each head boundary, so a deep ready-pool of wo
    matmuls fills the PE gaps left by the scores->exp->PV dependency chain;
    outputs stream from PSUM through ACT/DVE copies into bf16 staging and
    out via two wide DMAs per token tile.
  - B-phase SBUF pools are opened before the phase-A scratch pools so the
    first attention tiles never alias scratch whose release depends on the
    last RoPE chain.
"""
import math

import numpy as np
import ml_dtypes

import concourse.bass as bass
import concourse.tile as tile
from concourse import bacc, bass_isa, mybir
from concourse.bass_utils import run_bass_kernel_spmd
from concourse.masks import make_identity

B, S, DIM = 2, 2048, 4096
NH, NKV, HD = 32, 8, 128
BS = B * S
NCORES = 8
QH = NH // NCORES          # 4 Q heads per core
DQ = QH * HD               # 512
TCH = 512                  # token chunk
NCH = BS // TCH            # 8 chunks
NKT = DIM // 128           # 32 contraction tiles
P = 128

F32 = mybir.dt.float32
F32R = mybir.dt.float32r
BF16 = mybir.dt.bfloat16
AF = mybir.ActivationFunctionType
NPBF = ml_dtypes.bfloat16

_prog_cache = {}
LAST_RESULTS = None


def _build(variant):
    """variant: 'causal' | 'none' | 'general'"""
    nc = bacc.Bacc(None, target_bir_lowering=False)
    xT = nc.dram_tensor("xT", [DIM, BS], BF16, kind="ExternalInput")
    wq = nc.dram_tensor("wq", [P, QH * NKT * HD], BF16, kind="ExternalInput")
    wk = nc.dram_tensor("wk", [P, NKT * HD], BF16, kind="ExternalInput")
    wv = nc.dram_tensor("wv", [P, NKT * HD], BF16, kind="ExternalInput")
    wo = nc.dram_tensor("wo", [DQ, DIM], BF16, kind="ExternalInput")
    cosT = nc.dram_tensor("cosT", [64, S], BF16, kind="ExternalInput")
    sinT = nc.dram_tensor("sinT", [64, S], BF16, kind="ExternalInput")
    mblk = None
    emask = None
    if variant == "causal":
        mblk = nc.dram_tensor("mblk", [4 * P, TCH], BF16, kind="ExternalInput")
    elif variant == "general":
        emask = nc.dram_tensor("emaskT", [S, S], BF16, kind="ExternalInput")
    part = nc.dram_tensor("part", [BS, DIM], BF16, kind="ExternalOutput")

    with tile.TileContext(nc) as tc:
        with (
            tc.tile_pool(name="const", bufs=1) as constp,
            tc.tile_pool(name="big", bufs=1) as bigp,
        ):
            ident = constp.tile([P, P], BF16)
            make_identity(nc, ident)
            cos_sb = constp.tile([64, S], BF16)
            sin_sb = constp.tile([64, S], BF16)
            msk = []
            if variant == "causal":
                for j in range(4):
                    mj = constp.tile([P, TCH], BF16, name=f"msk{j}")
                    msk.append(mj)

            # persistent per-batch activations (bf16)
            KT_sb = [bigp.tile([P, S], BF16, name=f"KT{b}") for b in range(B)]
            Vtok = [bigp.tile([P, S], BF16, name=f"Vtok{b}") for b in range(B)]
            qt_sb = [[bigp.tile([P, S], BF16, name=f"qt{h}_{b}")
                      for b in range(B)] for h in range(QH)]
            w_sb = [bigp.tile([P, NKT * HD], BF16, name=f"w{m}")
                    for m in range(6)]
            wo_sb = [bigp.tile([P, DIM], BF16, name=f"wo{kk}")
                     for kk in range(4)]

            wsrc = [wq[:, m * NKT * HD:(m + 1) * NKT * HD] for m in range(QH)]
            wsrc += [wk[:, :], wv[:, :]]
            C1 = 4 * HD

            # B-phase SBUF pools open FIRST so their tiles never alias the
            # phase-A scratch (whose release depends on the tcn7 rope tail)
            ebp_cm = tc.tile_pool(name="ebp", bufs=1)
            ebp = ebp_cm.__enter__()
            mkp_cm = tc.tile_pool(name="mkp", bufs=1)
            mkp = mkp_cm.__enter__()
            obp_cm = tc.tile_pool(name="obp", bufs=1)
            obp = obp_cm.__enter__()

            # ---------------- Phase A: QKV projection + RoPE ----------------
            with (
                tc.tile_pool(name="xtp", bufs=1) as xtp,
                tc.tile_pool(name="rp", bufs=1) as rp,
                tc.tile_pool(name="psA", bufs=1, space="PSUM") as psA,
            ):
                for tcn in range(NCH):
                    b, cb = divmod(tcn, NCH // B)
                    acc = [psA.tile([P, TCH], F32, tag="acc", bufs=7,
                                    name=f"acc{m}_{tcn}") for m in range(6)]
                    for k in range(NKT):
                        xt = xtp.tile([P, TCH], BF16, tag="xt", bufs=9,
                                      name=f"xt_{tcn}_{k}")
                        nc.sync.dma_start(
                            xt[:], xT[k * P:(k + 1) * P,
                                      tcn * TCH:(tcn + 1) * TCH])
                        if tcn == 0 and k == 0:
                            for m in range(6):
                                eng = nc.sync if m % 2 == 0 else nc.gpsimd
                                eng.dma_start(w_sb[m][:, 0:C1],
                                              wsrc[m][:, 0:C1])
                        if tcn == 0 and k == 28:
                            nc.gpsimd.dma_start(cos_sb[:], cosT[:, :])
                            nc.gpsimd.dma_start(sin_sb[:], sinT[:, :])
                        if tcn == 1 and k == 0 and variant == "causal":
                            for j in range(4):
                                nc.gpsimd.dma_start(
                                    msk[j][:], mblk[j * P:(j + 1) * P, :])
                        if tcn == 0 and k in (0, 3, 6, 9):
                            lo = (4 + (k // 3) * 7) * HD
                            hi = min((4 + (k // 3 + 1) * 7) * HD, NKT * HD)
                            for m in range(6):
                                nc.gpsimd.dma_start(w_sb[m][:, lo:hi],
                                                    wsrc[m][:, lo:hi])

                        for m in range(6):
                            nc.tensor.matmul(
                                acc[m][:], w_sb[m][:, k * HD:(k + 1) * HD],
                                xt[:], start=(k == 0), stop=(k == NKT - 1))

                    cs = cos_sb[:, cb * TCH:(cb + 1) * TCH]
                    sn = sin_sb[:, cb * TCH:(cb + 1) * TCH]
                    vch = rp.tile([P, TCH], BF16, tag="vch", bufs=2,
                                  name=f"vch_{tcn}")
                    nc.scalar.copy(vch[:], acc[5][:])
                    slos, shis = [], []
                    for m in range(5):
                        slo = rp.tile([64, TCH], BF16, tag="slo", bufs=3,
                                      name=f"slo{m}_{tcn}")
                        shi = rp.tile([64, TCH], BF16, tag="shi", bufs=3,
                                      name=f"shi{m}_{tcn}")
                        if m % 2 == 0:
                            nc.scalar.copy(slo[:], acc[m][0:64, :])
                            nc.vector.tensor_copy(shi[:], acc[m][64:P, :])
                        else:
                            nc.vector.tensor_copy(slo[:], acc[m][0:64, :])
                            nc.scalar.copy(shi[:], acc[m][64:P, :])
                        slos.append(slo)
                        shis.append(shi)
                    tp4 = psA.tile([P, 4 * P], BF16, tag="tp", bufs=1,
                                   name=f"tp_{tcn}")
                    for j in range(TCH // P):
                        nc.tensor.transpose(
                            tp4[:, j * P:(j + 1) * P],
                            vch[:, j * P:(j + 1) * P], ident[:])
                    nc.vector.tensor_copy(
                        Vtok[b][:, cb * TCH:(cb + 1) * TCH], tp4[:])
                    for m in range(5):
                        slo, shi = slos[m], shis[m]
                        dst = qt_sb[m][b] if m < QH else KT_sb[b]
                        o_lo = dst[0:64, cb * TCH:(cb + 1) * TCH]
                        o_hi = dst[64:P, cb * TCH:(cb + 1) * TCH]
                        tA = rp.tile([64, TCH], BF16, tag="tA", bufs=2,
                                     name=f"tA{m}_{tcn}")
                        tB = rp.tile([64, TCH], BF16, tag="tB", bufs=2,
                                     name=f"tB{m}_{tcn}")
                        nc.vector.tensor_mul(tA[:], slo[:], cs)
                        nc.vector.tensor_mul(tB[:], shi[:], sn)
                        nc.vector.tensor_sub(o_lo, tA[:], tB[:])
                        tC = rp.tile([64, TCH], BF16, tag="tC", bufs=2,
                                     name=f"tC{m}_{tcn}")
                        tD = rp.tile([64, TCH], BF16, tag="tD", bufs=2,
                                     name=f"tD{m}_{tcn}")
                        nc.vector.tensor_mul(tC[:], slo[:], sn)
                        nc.vector.tensor_mul(tD[:], shi[:], cs)
                        nc.vector.tensor_add(o_hi, tC[:], tD[:])


            # ------------- Phase B+C merged per (batch, chunk) -------------
            with (
                tc.tile_pool(name="psB", bufs=1, space="PSUM") as psB,
            ):
                for kk in range(4):
                    nc.sync.dma_start(wo_sb[kk][:],
                                      wo[kk * P:(kk + 1) * P, :])
                pending_wo = []
                for b in range(B):
                    for sc in (0, 1, 2, 3):
                        ntt = 4 * sc + 4 if variant == "causal" else 16
                        o_g = [obp.tile([P, TCH], BF16, tag=f"og{h}",
                                        bufs=3, name=f"og_{b}_{sc}_{h}")
                               for h in range(QH)]
                        for h in range(QH):
                            if len(pending_wo) > 3:
                                pending_wo.pop(0)()
                            o_ps = psB.tile([P, TCH], F32, tag="o", bufs=2,
                                            name=f"o_{b}_{sc}_{h}")
                            E_ab = [ebp.tile([P, TCH], BF16, tag=f"ea{par}",
                                             bufs=2,
                                             name=f"ea{par}_{b}_{sc}_{h}")
                                    for par in range(2)]
                            for tt in range(ntt):
                                sc_ps = psB.tile([P, TCH], F32, tag="sc",
                                                 bufs=3,
                                                 name=f"s_{b}_{sc}_{h}_{tt}")
                                nc.tensor.matmul(
                                    sc_ps[:],
                                    KT_sb[b][:, tt * P:(tt + 1) * P],
                                    qt_sb[h][b][:, sc * TCH:(sc + 1) * TCH],
                                    start=True, stop=True)
                                masked = (variant == "general") or (
                                    variant == "causal" and tt >= 4 * sc)
                                et = ebp.tile([P, TCH], BF16, tag="et",
                                              bufs=6,
                                              name=f"et_{b}_{sc}_{h}_{tt}")
                                if masked:
                                    etm = ebp.tile(
                                        [P, TCH], BF16, tag="etm", bufs=2,
                                        name=f"em_{b}_{sc}_{h}_{tt}")
                                    nc.scalar.activation(etm[:], sc_ps[:],
                                                         AF.Exp)
                                    if variant == "causal":
                                        mt = msk[tt - 4 * sc][:]
                                    else:
                                        mg = mkp.tile(
                                            [P, TCH], BF16, tag="mg", bufs=3,
                                            name=f"mg_{b}_{sc}_{h}_{tt}")
                                        nc.sync.dma_start(
                                            mg[:],
                                            emask[tt * P:(tt + 1) * P,
                                                  sc * TCH:(sc + 1) * TCH])
                                        mt = mg[:]
                                    nc.vector.tensor_mul(et[:], etm[:], mt)
                                else:
                                    nc.scalar.activation(et[:], sc_ps[:],
                                                         AF.Exp)
                                ea = E_ab[tt % 2]
                                if tt < 2:
                                    nc.vector.tensor_copy(ea[:], et[:])
                                else:
                                    nc.vector.tensor_add(ea[:], ea[:], et[:])
                                nc.tensor.matmul(
                                    o_ps[:], Vtok[b][:, tt * P:(tt + 1) * P],
                                    et[:], start=(tt == 0),
                                    stop=(tt == ntt - 1))
                            e_sum = ebp.tile([P, TCH], BF16, tag="es",
                                             bufs=1, name=f"es_{b}_{sc}_{h}")
                            nc.vector.tensor_add(e_sum[:], E_ab[0][:],
                                                 E_ab[1][:])
                            srec = obp.tile([P, TCH], F32, tag="sr", bufs=1,
                                            name=f"sr_{b}_{sc}_{h}")
                            nc.gpsimd.partition_all_reduce(
                                srec[:], e_sum[:], P, bass_isa.ReduceOp.add)
                            rec = obp.tile([P, TCH], F32, tag="rec", bufs=1,
                                           name=f"rec_{b}_{sc}_{h}")
                            nc.vector.reciprocal(rec[:], srec[:])
                            nc.vector.tensor_mul(o_g[h][:], o_ps[:], rec[:])

                        # wo projection for this chunk, deferred; one
                        # closure per token tile for fine-grained interleave
                        def make_wo(b=b, sc=sc, o_g=o_g, tj=0):
                            def emit():
                                last = (b == B - 1 and sc == 3 and tj >= 2)
                                nflush = 4 if last else 2
                                if True:
                                    tt = 4 * sc + tj
                                    for half in range(nflush):
                                        fw = DIM // nflush
                                        ob = obp.tile(
                                            [P, fw], BF16, tag="ob",
                                            bufs=3,
                                            name=f"ob_{b}_{tt}_{half}")
                                        for nj in range(4 // (nflush // 2)):
                                            nn = half * (4 // (nflush // 2)) + nj
                                            pp = psB.tile(
                                                [P, TCH], F32, tag="pp",
                                                bufs=3,
                                                name=f"pp_{b}_{tt}_{nn}")
                                            for kk in range(4):
                                                nc.tensor.matmul(
                                                    pp[:],
                                                    o_g[kk][:,
                                                            tj * P:
                                                            (tj + 1) * P],
                                                    wo_sb[kk][:,
                                                              nn * TCH:
                                                              (nn + 1) * TCH],
                                                    start=(kk == 0),
                                                    stop=(kk == 3))
                                            dst = ob[:, nj * TCH:
                                                     (nj + 1) * TCH]
                                            if nn % 2 == 0:
                                                nc.scalar.copy(dst, pp[:])
                                            else:
                                                nc.vector.tensor_copy(dst,
                                                                      pp[:])
                                        nc.sync.dma_start(
                                            part[b * S + tt * P:
                                                 b * S + (tt + 1) * P,
                                                 half * fw:(half + 1) * fw],
                                            ob[:])
                            return emit
                        for tj in range(4):
                            pending_wo.append(make_wo(tj=tj))
                for fn_ in pending_wo:
                    fn_()
                pending_wo = []

                obp_cm.__exit__(None, None, None)
                mkp_cm.__exit__(None, None, None)
                ebp_cm.__exit__(None, None, None)

    nc.compile()
    return nc


def _get_prog(variant):
    if variant not in _prog_cache:
        _prog_cache[variant] = _build(variant)
    return _prog_cache[variant]


def prepare(inputs):
    """Host-side sharding prep: returns (variant, program, per-core input maps)."""
    x = np.asarray(inputs["x"], dtype=np.float32)
    wq = np.asarray(inputs["wq"], dtype=np.float32)
    wk = np.asarray(inputs["wk"], dtype=np.float32)
    wv = np.asarray(inputs["wv"], dtype=np.float32)
    wo = np.asarray(inputs["wo"], dtype=np.float32)
    fc = np.asarray(inputs["freqs_cos"], dtype=np.float32)
    fs = np.asarray(inputs["freqs_sin"], dtype=np.float32)
    mask = np.asarray(inputs["mask"], dtype=np.float32)

    xT = np.ascontiguousarray(x.reshape(BS, DIM).T).astype(NPBF)
    perm = np.concatenate([np.arange(0, HD, 2), np.arange(1, HD, 2)])
    wq_p = (wq.reshape(DIM, NH, HD)[:, :, perm] / math.sqrt(HD))
    wk_p = wk.reshape(DIM, NKV, HD)[:, :, perm]
    cosT = np.ascontiguousarray(fc.T).astype(NPBF)
    sinT = np.ascontiguousarray(fs.T).astype(NPBF)

    if not mask.any():
        variant = "none"
    else:
        il, jl = np.tril_indices(S)
        iu, ju = np.triu_indices(S, 1)
        if np.all(mask[il, jl] == 0.0) and np.all(mask[iu, ju] <= -1e8):
            variant = "causal"
        else:
            variant = "general"

    mblk = None
    emaskT = None
    if variant == "causal":
        t = np.arange(4 * P)[:, None]
        q = np.arange(TCH)[None, :]
        mblk = (q >= t).astype(NPBF)
    elif variant == "general":
        with np.errstate(under="ignore", over="ignore"):
            emaskT = np.ascontiguousarray(np.exp(mask).T).astype(NPBF)

    nc = _get_prog(variant)

    in_maps = []
    for c in range(NCORES):
        wqc = wq_p[:, c * QH:(c + 1) * QH, :]                    # [DIM,QH,HD]
        wqc = np.ascontiguousarray(
            wqc.reshape(NKT, P, QH, HD).transpose(1, 2, 0, 3)
            .reshape(P, QH * NKT * HD)).astype(NPBF)
        wkc = np.ascontiguousarray(
            wk_p[:, c, :].reshape(NKT, P, HD).transpose(1, 0, 2)
            .reshape(P, NKT * HD)).astype(NPBF)
        wvc = np.ascontiguousarray(
            wv[:, c * HD:(c + 1) * HD].reshape(NKT, P, HD).transpose(1, 0, 2)
            .reshape(P, NKT * HD)).astype(NPBF)
        m = {
            "xT": xT,
            "wq": wqc,
            "wk": wkc,
            "wv": wvc,
            "wo": np.ascontiguousarray(
                wo[c * DQ:(c + 1) * DQ, :]).astype(NPBF),
            "cosT": cosT,
            "sinT": sinT,
        }
        if variant == "causal":
            m["mblk"] = mblk
        elif variant == "general":
            m["emaskT"] = emaskT
        in_maps.append(m)
    return variant, nc, in_maps


def kernel(**inputs):
    global LAST_RESULTS
    variant, nc, in_maps = prepare(inputs)
    out = None
    for attempt in range(3):
        res = run_bass_kernel_spmd(nc, in_maps, core_ids=list(range(NCORES)))
        LAST_RESULTS = res
        out = np.zeros((BS, DIM), dtype=np.float64)
        ok = True
        for c in range(NCORES):
            p = np.asarray(res.results[c]["part"], dtype=np.float64)
            # flaky-execution guard: a healthy partial is finite, nonzero,
            # and O(1)-scale; garbage shows up as huge values or all zeros
            if not np.isfinite(p).all() or np.abs(p).max() > 1e3 \
                    or np.abs(p).max() == 0.0:
                ok = False
            out += p
        if ok:
            break
    return out.reshape(B, S, DIM).astype(np.float32)
